# revision 14
# baseline (speedup 1.0000x reference)
"""Trainium2 Bass kernel for the Convpass-swin hypernet-fuse adapter module.

Data-parallel over batch: 32 samples -> 8 cores x 4 samples; small weights
replicated. All PE matmuls run in bf16 (fp32 PSUM accumulate); fp32 matmuls
on this part run ~3x slower per row and double the DMA traffic, so operands
are pre-cast on the host.

Per-core dataflow (R = 4*28*28 = 3136 spatial rows, C=768, D=EMB=64):
  1. x arrives host-pre-transposed and chunked: xstk[chunk, 128, ktile, 392]
     (chunk = half-sample of 14 rows), so no PE transposes are needed and each
     chunk is one DMA with 4704B-contiguous per-partition descriptors.
     Chunks alternate between the sync and scalar DMA queues.
  2. Stacked matmul (K=C) computes meta1 and adapter-down together per chunk:
     PSUM [128, 392]; rows 0:64 -> ACT Relu(+b1) with accum_out => per-chunk
     sum of h; rows 64:128 -> qgelu via Sigmoid + DVE (x+b)*sig, written twice
     into a zero-padded bf16 [128, 4, 30, 30] buffer (upper rows shifted one
     column left so conv taps (dh,0)/(dh,1) pair into one K=128 matmul).
  3. prompt = (sum_h/784) @ w2.T (+ b2 + layer_emb) -> fused, replicated into
     a block-diagonal [128, 128] bf16 stationary (16 replicas of 4 cols/half).
  4. Hypernet: hwt (bf16, host-permuted j' = (dw, d_in, dh, d_out), even/odd
     512-chunks stacked on partition halves) loads as 6 resident SBUF tiles
     on the gpsimd queue, gated behind x chunk 3 so x keeps full HBM
     bandwidth first. One K=128 matmul per [128, 512] tile; ACT/DVE cast
     PSUM->bf16 staging; 2 strided DMAs per 2-tile group bounce to a DRAM
     scratch whose read-back view is 384B-contiguous per partition.
  5. Conv per sample: 2 contiguous DMAs fetch tap-paired weight tiles +
     hyper-bias add. Per (sample, half): 3 paired K=128 matmuls stream the
     full contiguous 30-wide padded rows (garbage output cols masked later)
     + 3 single K=64 matmuls -> PSUM [64, 14, 30]; qgelu -> yg [65, R] bf16
     (row 64 = ones).
  6. Up-projection interleaved per sample: out row-tiles fully covered by
     completed samples are emitted immediately (matmul -> cast copy
     alternating ACT/DVE -> DMA out alternating sync/scalar queues), so the
     4.8MB output write overlaps the remaining conv/up compute.
"""

import sys

sys.path.insert(0, "/opt/trn_rl_repo")

import ml_dtypes
import numpy as np

import concourse.bass as bass
import concourse.tile as tile
from concourse import bacc, mybir
from concourse.bass_utils import run_bass_kernel_spmd

F32 = mybir.dt.float32
BF16 = mybir.dt.bfloat16
AF = mybir.ActivationFunctionType
OP = mybir.AluOpType

B, H, W, C, D, EMB = 32, 28, 28, 768, 64, 64
NCORES = 8
BL = B // NCORES            # samples per core
R = BL * H * W              # 3136 rows per core
RT = (R + 127) // 128       # 25 row tiles
HP, WP = H + 2, W + 2       # padded 30x30
JTOT = D * D * 9            # 36864 hypernet outputs per sample
NCH = JTOT // 512           # 72 chunks of 512
NHT = NCH // 2              # 36 hypernet weight tiles [128, 512]
HTG = 2                     # hypernet tiles per staging group
NHG = 6                     # hwt load granularity: 6 DMAs of 6 tiles
NB = 392                    # half-sample chunk (14 rows of 28)
NBF = 14 * WP               # full-row conv chunk incl pad cols (420)
ONES_BF16_PAIR = 1.0019378662109375  # f32 whose bits are two bf16 1.0s

TRACE = False               # set True (e.g. from test.py) to capture a profile
LAST_EXEC_NS = None         # filled from the profile when TRACE is on

_cached = {}


def _build_program():
    nc = bacc.Bacc("TRN2", target_bir_lowering=False, debug=False)

    xstk = nc.declare_dram_parameter("xstk", [8 * 128, 6 * NB], BF16, isOutput=False).ap()
    wstk = nc.declare_dram_parameter("wstk", [128, 6 * 128], BF16, isOutput=False).ap()
    brelu = nc.declare_dram_parameter("brelu", [64, 1], F32, isOutput=False).ap()
    bsilu = nc.declare_dram_parameter("bsilu", [64, 1], F32, isOutput=False).ap()
    dwb = nc.declare_dram_parameter("dwb", [64, 1], F32, isOutput=False).ap()
    w2t = nc.declare_dram_parameter("w2t", [64, 64], BF16, isOutput=False).ap()
    fbv = nc.declare_dram_parameter("fbv", [64, 1], F32, isOutput=False).ap()
    hwt = nc.declare_dram_parameter("hwt", [128, NHT * 512], BF16, isOutput=False).ap()
    hbp2 = nc.declare_dram_parameter("hbp2", [128, 192], BF16, isOutput=False).ap()
    hbp3 = nc.declare_dram_parameter("hbp3", [64, 192], BF16, isOutput=False).ap()
    upw = nc.declare_dram_parameter("upw", [65, C], BF16, isOutput=False).ap()
    out = nc.declare_dram_parameter("out", [R, C], BF16, isOutput=True).ap()

    with tile.TileContext(nc) as tc, \
         tc.tile_pool(name="consts", bufs=1) as cpool, \
         tc.tile_pool(name="xin", bufs=4) as xinpool, \
         tc.tile_pool(name="work", bufs=2) as wpool, \
         tc.tile_pool(name="cwsb", bufs=2) as cwsbpool, \
         tc.tile_pool(name="cwtp", bufs=2) as cwtpool, \
         tc.tile_pool(name="outp", bufs=4) as outpool, \
         tc.tile_pool(name="dram", bufs=1, space="DRAM") as dpool:

        # ---------- standing buffers / constants (gpsimd queue) ----------
        wstk_sb = cpool.tile([128, 768], BF16, tag="wstk")
        nc.gpsimd.dma_start(out=wstk_sb[:], in_=wstk)
        w2t_sb = cpool.tile([64, 64], BF16, tag="w2t")
        nc.gpsimd.dma_start(out=w2t_sb[:], in_=w2t)
        brelu_sb = cpool.tile([64, 1], F32, tag="brelu")
        nc.gpsimd.dma_start(out=brelu_sb[:], in_=brelu)
        bsilu_sb = cpool.tile([64, 1], F32, tag="bsilu")
        nc.gpsimd.dma_start(out=bsilu_sb[:], in_=bsilu)
        dwb_sb = cpool.tile([64, 1], F32, tag="dwb")
        nc.gpsimd.dma_start(out=dwb_sb[:], in_=dwb)
        fb_sb = cpool.tile([64, 1], F32, tag="fbv")
        nc.gpsimd.dma_start(out=fb_sb[:], in_=fbv)
        upw_sb = cpool.tile([65, C], BF16, tag="upw")
        nc.gpsimd.dma_start(out=upw_sb[:], in_=upw)
        hbp2_sb = cpool.tile([128, 192], BF16, tag="hbp2")
        nc.gpsimd.dma_start(out=hbp2_sb[:], in_=hbp2)
        hbp3_sb = cpool.tile([64, 192], BF16, tag="hbp3")
        nc.gpsimd.dma_start(out=hbp3_sb[:], in_=hbp3)

        # +2 spare cols: the conv dw=2 matmuls read 2 cols past each row end
        s1pad = cpool.tile([128, BL * HP * WP + 2], BF16, tag="s1pad")
        nc.vector.memset(s1pad[:].bitcast(F32), 0.0)
        mha_sb = cpool.tile([64, 2 * BL], F32, tag="mha")
        mh_sb = cpool.tile([64, BL], BF16, tag="mh")
        fused_sb = cpool.tile([128, 128], BF16, tag="fused")
        nc.vector.memset(fused_sb[:].bitcast(F32), 0.0)
        yg_sb = cpool.tile([65, R], BF16, tag="yg")
        nc.vector.memset(yg_sb[64:65, :].bitcast(F32), ONES_BF16_PAIR)
        gate_sb = cpool.tile([64, 1], F32, tag="gate")
        cw_dram = dpool.tile([BL, JTOT], BF16, tag="cw")

        s1v = s1pad[:, 0:BL * HP * WP].rearrange(
            "p (b h w) -> p b h w", b=BL, h=HP, w=WP
        )

        # ---------- phase A: stacked meta1+down, prompt ----------
        with tc.tile_pool(name="stkps", bufs=2, space="PSUM") as stkpool:

            xq = [nc.sync, nc.scalar]
            for cix in range(8):
                b, hc = divmod(cix, 2)
                xc = xinpool.tile([128, 6 * NB], BF16, tag="xin")
                xq[cix % 2].dma_start(
                    out=xc[:], in_=xstk[cix * 128:(cix + 1) * 128, :],
                )
                if cix == 3:
                    # gate: release the hwt loads only once x streaming is
                    # well underway, so x keeps full HBM bandwidth early
                    # (reads chunk 2's accum column, whose writer precedes
                    # this in program order)
                    nc.gpsimd.tensor_copy(out=gate_sb[:], in_=mha_sb[:, 2:3])
                    hwt_tiles = []
                    for j in range(NHG):
                        hwt_sb = cpool.tile([128, NHT * 512 // NHG], BF16,
                                            tag=f"hwt{j}")
                        nc.gpsimd.dma_start(
                            out=hwt_sb[:],
                            in_=hwt[:, j * NHT * 512 // NHG:(j + 1) * NHT * 512 // NHG],
                        )
                        hwt_tiles.append(hwt_sb)
                ps = stkpool.tile([128, NB], F32, tag="stk", name="ps")
                for kt in range(6):
                    nc.tensor.matmul(
                        ps[:],
                        lhsT=wstk_sb[:, kt * 128:(kt + 1) * 128],
                        rhs=xc[:, kt * NB:(kt + 1) * NB],
                        start=(kt == 0),
                        stop=(kt == 5),
                    )
                hsc = wpool.tile([64, NB], F32, tag="hsc", name="hsc")
                nc.scalar.activation(
                    hsc[:], ps[0:64, :], AF.Relu,
                    bias=brelu_sb[:], accum_out=mha_sb[:, cix:cix + 1],
                )
                sg1 = wpool.tile([64, NB], F32, tag="sg1", name="sg1")
                nc.scalar.activation(
                    sg1[:], ps[64:128, :], AF.Sigmoid,
                    bias=bsilu_sb[:], scale=1.702,
                )
                ps3 = ps[64:128, :].rearrange("p (h w) -> p h w", h=14, w=W)
                sg13 = sg1[:].rearrange("p (h w) -> p h w", h=14, w=W)
                h0 = hc * 14 + 1
                nc.vector.scalar_tensor_tensor(
                    out=s1v[0:64, b, h0:h0 + 14, 1:W + 1],
                    in0=ps3, scalar=dwb_sb[:], in1=sg13,
                    op0=OP.add, op1=OP.mult,
                )
                nc.vector.scalar_tensor_tensor(
                    out=s1v[64:128, b, h0:h0 + 14, 0:W],
                    in0=ps3, scalar=dwb_sb[:], in1=sg13,
                    op0=OP.add, op1=OP.mult,
                )
                if hc == 1:
                    mhv = mha_sb[:].rearrange("p (b h) -> p b h", b=BL)
                    nc.vector.tensor_add(
                        mh_sb[:, b:b + 1], mhv[:, b, 0:1], mhv[:, b, 1:2]
                    )

            pp = stkpool.tile([64, BL], F32, tag="stk")
            nc.tensor.matmul(
                pp[:], lhsT=w2t_sb[:], rhs=mh_sb[:], start=True, stop=True,
            )
            nc.scalar.activation(fused_sb[0:64, 0:BL], pp[:], AF.Identity, bias=fb_sb[:])
            nc.scalar.activation(
                fused_sb[64:128, 64:64 + BL], pp[:], AF.Identity, bias=fb_sb[:]
            )
            w = BL
            while w < 64:
                nc.vector.tensor_copy(
                    out=fused_sb[0:64, w:2 * w], in_=fused_sb[0:64, 0:w]
                )
                nc.scalar.copy(
                    out=fused_sb[64:128, 64 + w:64 + 2 * w],
                    in_=fused_sb[64:128, 64:64 + w],
                )
                w *= 2

        # ---------- phase B: hypernet, conv, up-projection ----------
        # cw_dram[b, j'], j' = (g, k, par, s): chunk c = 2*(HTG*g + k) + par
        cwg = cw_dram[:].rearrange(
            "b (g k par s) -> g par b k s", g=NHT // HTG, k=HTG, par=2, s=512
        )
        # conv weight fetch view: j' = ((dw, di), dh, do); per partition the
        # (dh, do) block is one contiguous 384B run.
        cwt3 = cw_dram[:].rearrange(
            "b (dwdi f) -> b dwdi f", dwdi=3 * D, f=3 * D
        )

        with tc.tile_pool(name="cwps", bufs=4, space="PSUM") as cwpool:
            for g in range(NHT // HTG):
                cw_sb = cwsbpool.tile([128, HTG * 512], BF16, tag="cwsb")
                for k in range(HTG):
                    ti = g * HTG + k
                    cps = cwpool.tile([128, 512], F32, tag="cw")
                    tpg = NHT // NHG
                    nc.tensor.matmul(
                        cps[:], lhsT=fused_sb[:],
                        rhs=hwt_tiles[ti // tpg][:, (ti % tpg) * 512:(ti % tpg + 1) * 512],
                        start=True, stop=True,
                    )
                    if ti % 2 == 0:
                        nc.scalar.copy(out=cw_sb[:, k * 512:(k + 1) * 512], in_=cps[:])
                    else:
                        nc.vector.tensor_copy(out=cw_sb[:, k * 512:(k + 1) * 512], in_=cps[:])
                cwv = cw_sb[:].rearrange("p (k s) -> p k s", k=HTG)
                nc.gpsimd.dma_start(out=cwg[g, 0], in_=cwv[0:BL])
                nc.gpsimd.dma_start(out=cwg[g, 1], in_=cwv[64:64 + BL])

        with tc.tile_pool(name="cvps", bufs=3, space="PSUM") as cvpool, \
             tc.tile_pool(name="upps", bufs=4, space="PSUM") as uppool:

            done_rt = 0
            copy_rot = 0
            copy_engines = (nc.scalar.copy, nc.vector.tensor_copy)
            outq = [nc.sync, nc.scalar]
            for b in range(BL):
                cwp_sb = cwtpool.tile([128, 192], BF16, tag="cwp")
                nc.gpsimd.dma_start(out=cwp_sb[:], in_=cwt3[b, 0:128])
                nc.vector.tensor_add(cwp_sb[:], cwp_sb[:], hbp2_sb[:])
                cws_sb = cwtpool.tile([64, 192], BF16, tag="cws")
                nc.gpsimd.dma_start(out=cws_sb[:], in_=cwt3[b, 128:192])
                nc.vector.tensor_add(cws_sb[:], cws_sb[:], hbp3_sb[:])
                boff = b * HP * WP
                for hc in range(2):
                    cvp = cvpool.tile([64, NBF], F32, tag="cv")
                    cvp3 = cvp[:].rearrange("p (h w) -> p h w", h=14, w=WP)
                    for dh in range(3):
                        c0 = boff + (hc * 14 + dh) * WP
                        nc.tensor.matmul(
                            cvp[:],
                            lhsT=cwp_sb[:, dh * 64:(dh + 1) * 64],
                            rhs=s1pad[:, c0:c0 + NBF],
                            start=(dh == 0), stop=False,
                        )
                        nc.tensor.matmul(
                            cvp[:],
                            lhsT=cws_sb[:, dh * 64:(dh + 1) * 64],
                            rhs=s1pad[0:64, c0 + 2:c0 + 2 + NBF],
                            start=False, stop=(dh == 2),
                        )
                    sg2 = wpool.tile([64, NBF], F32, tag="sg2")
                    nc.scalar.activation(sg2[:], cvp[:], AF.Sigmoid, scale=1.702)
                    sg23 = sg2[:].rearrange("p (h w) -> p h w", h=14, w=WP)
                    ygv = yg_sb[0:64, b * 784 + hc * NB: b * 784 + (hc + 1) * NB]
                    nc.vector.tensor_mul(
                        ygv.rearrange("p (h w) -> p h w", h=14, w=W),
                        cvp3[:, :, 0:W], sg23[:, :, 0:W],
                    )

                upto = RT if b == BL - 1 else ((b + 1) * 784) // 128
                for rt in range(done_rt, upto):
                    r0 = rt * 128
                    rsz = min(128, R - r0)
                    osb = outpool.tile([128, C], BF16, tag="osb", name="osb")
                    for (n0, nsz) in ((0, 384), (384, 384)):
                        upp = uppool.tile([128, 384], F32, tag="up", name="upp")
                        nc.tensor.matmul(
                            upp[:rsz, :nsz],
                            lhsT=yg_sb[:, r0:r0 + rsz],
                            rhs=upw_sb[:, n0:n0 + nsz],
                            start=True, stop=True,
                        )
                        copy_engines[copy_rot % 2](
                            out=osb[:rsz, n0:n0 + nsz], in_=upp[:rsz, :nsz]
                        )
                        copy_rot += 1
                    outq[rt % 2].dma_start(out=out[r0:r0 + rsz, :], in_=osb[:rsz, :])
                done_rt = upto

    nc.compile()
    return nc


def _prep_host(inputs):
    f = lambda a: np.ascontiguousarray(np.asarray(a, dtype=np.float32))
    bf = lambda a: np.ascontiguousarray(np.asarray(a).astype(ml_dtypes.bfloat16))
    x = f(inputs["x"])
    meta_w1, meta_b1 = f(inputs["meta_w1"]), f(inputs["meta_b1"])
    meta_w2, meta_b2 = f(inputs["meta_w2"]), f(inputs["meta_b2"])
    layer_emb = f(inputs["layer_emb"])
    hyper_w, hyper_b = f(inputs["hyper_w"]), f(inputs["hyper_b"])
    down_w, down_b = f(inputs["down_w"]), f(inputs["down_b"])
    up_w, up_b = f(inputs["up_w"]), f(inputs["up_b"])

    # [C, 128] -> partition-major [128, 6*128]: wstk[p, t*128+m] = W.T[t*128+p, m]
    wst = np.concatenate([meta_w1, down_w], axis=0).T  # [C, 128]
    wstk = bf(wst.reshape(6, 128, 128).transpose(1, 0, 2).reshape(128, 768))
    brelu = meta_b1.reshape(64, 1)
    bsilu = (1.702 * down_b).reshape(64, 1)
    dwb = down_b.reshape(64, 1)
    w2t = bf(meta_w2.T / 784.0)  # lhsT[o,p] = w2[p,o]/HW
    fbv = (meta_b2 + layer_emb).reshape(64, 1)

    # hyper_w [j, e], j = (do, di, kh, kw)  ->  HWTperm [e, j'], j' = (kw, di, kh, do)
    hw5 = hyper_w.reshape(D, D, 3, 3, EMB)            # do, di, kh, kw, e
    hwtp = np.ascontiguousarray(hw5.transpose(4, 3, 1, 2, 0)).reshape(EMB, JTOT)
    # stack even/odd 512-chunks on partition halves -> [128, NHT*512]
    hwt = bf(
        hwtp.reshape(EMB, NHT, 2, 512).transpose(2, 0, 1, 3).reshape(128, NHT * 512)
    )
    # hyper bias in the two conv-weight tile layouts: [dw, di, dh, do]
    hb4 = hyper_b.reshape(D, D, 3, 3).transpose(3, 1, 2, 0)
    hbp2 = bf(hb4[0:2].reshape(128, 192))
    hbp3 = bf(hb4[2].reshape(64, 192))

    upw = bf(np.concatenate([up_w.T, up_b.reshape(1, C)], axis=0))  # [65, C]

    shared = dict(wstk=wstk, brelu=brelu, bsilu=bsilu, dwb=dwb, w2t=w2t,
                  fbv=fbv, hwt=hwt, hbp2=hbp2, hbp3=hbp3, upw=upw)
    in_maps = []
    for k in range(NCORES):
        m = dict(shared)
        # xT chunks, partition-major: [8 chunks, 128 c-part, 6 ktiles, 392]
        xr = x[k * BL:(k + 1) * BL].reshape(R, C)
        m["xstk"] = bf(
            xr.reshape(8, NB, 6, 128).transpose(0, 3, 2, 1).reshape(8 * 128, 6 * NB)
        )
        in_maps.append(m)
    return in_maps


def kernel(**inputs) -> np.ndarray:
    if "nc" not in _cached:
        _cached["nc"] = _build_program()
    nc = _cached["nc"]
    in_maps = _prep_host(inputs)
    res = run_bass_kernel_spmd(nc, in_maps, list(range(NCORES)), trace=TRACE)
    global LAST_EXEC_NS
    if TRACE and res.exec_time_ns is not None:
        LAST_EXEC_NS = res.exec_time_ns
        print(f"HW exec time: {res.exec_time_ns} ns")
    outs = [
        np.asarray(res.results[k]["out"]).astype(np.float32).reshape(BL, H, W, C)
        for k in range(NCORES)
    ]
    return np.concatenate(outs, axis=0)
